# revision 12
# baseline (speedup 1.0000x reference)
"""DiceLoss partial-sum kernel for Trainium2 (8 NeuronCores, data-parallel).

Computes, for input/target of shape (32, 1, 1024, 1024) fp32:
    bin   = (input > 0.5) ? 1.0 : 0.0
    loss1 = 2 * sum(bin * target)
    loss2 = sum(bin) + sum(target)
and returns (loss1, loss2) as fp32 scalars (same structure as the reference).

Sharding: batch dim N=32 is split 4-per-core across 8 cores. Each core
streams its 16 MiB input + 16 MiB target shard through SBUF as [128, F]
fp32 tiles via HWDGE DMA on the sync queue. The problem is HBM-bound
(~429 GB/s/core sustained over 16 HW DMA engines = ~9.8 us per 4096-wide
tile pair; 8192-wide loads' 32 KB descriptors gained only ~1%), so the
design keeps the DMA descriptor queue fed but NOT overflowed, and every
engine's per-tile work under the DMA pace:
  loads:  7x4096 + 2048 + 2x1024 wide tiles through a 4-slot SBUF ring.
          Slot reuse (tile t waits consumers of t-4) paces descriptor
          submission: with the dve/act compute split the waits fire
          ~10 us or more before the backend drains the queue, so DMA
          never starves - but they DO hold the tail tiles' descriptors
          back enough that the HWDGE descriptor ring never fills. (A
          variant issuing the taper loads with no consumer coupling
          overflowed the ring: the dma_starts themselves blocked
          4-10 us each in descriptor generation, and the tail data
          slipped ~15 us.)
  vector: every tile: STT (in>0.5)*tgt, accum -> loss1 col; on "dve"
          tiles also STT (in>0.5)+tgt, accum -> loss2 col (exact).
          STT sink is a single PSUM buffer (write-only; self-waits
          serialize retirement, and PSUM use frees SBUF).
  scalar: on "act" tiles: Copy(tgt) accum -> tgt col and Sign(1-2*in)
          accum -> sign col; bin count recovered on host as
          (count - S')/2, exact up to elements equal to 0.5 (~1e-8 rel).
  gpsimd: zeroes the stats tile once at start.
The dve/act alternation keeps vector at ~68% and scalar at ~41% of the
DMA pace on average, so the pipeline tolerates the ~20% engine-clock
p-state throttling this part sometimes shows. The tile taper (last
tiles 2048/1024/1024) keeps the post-last-byte compute tail small; the
taper's loss2 modes (act, dve, dve) balance the two engines' serial
tail chains (~90.3 vs ~91.4 us on a 94 us run). Stats are per-tile
column triplets; tiles 0..nt-3 DMA out overlapped with the taper
compute, then a [128, 6] DMA ships the last two triplets. Final
reduction over cores/partitions/tiles happens on the host in float64.

Measured (NTFF, core 0): 93988 ns, rel err 0.0. Budget: ~0.5 us to
first dma_start + ~80 us DMA-bound (incl. ramp + taper drain) +
~2.6 us arrival-limited compute tail + ~1.3 us end chain + ~7.3 us
NEFF exit (fixed 253-semaphore reset sweep, Tensor-sequencer-bound,
emitted by the NEFF lowering - not controllable from kernel code).
Some runs show ~20% slower engine clocks (STT 4432 -> 5318 ns) with
unchanged DMA; those runs measure ~8-12 us slower end to end.
"""

from contextlib import ExitStack

import numpy as np

try:
    import concourse.bass  # noqa: F401
except ImportError:  # pragma: no cover - path fallback for bare containers
    import sys

    for _p in ("/opt/trn_rl_repo", "/root/.axon_site/_ro/trn_rl_repo"):
        if _p not in sys.path:
            sys.path.insert(0, _p)

import concourse.bacc as bacc
import concourse.mybir as mybir
from concourse.bass_utils import run_bass_kernel_spmd

N_CORES = 8
FULL_SHAPE = (32, 1, 1024, 1024)
FULL_ELEMS = 32 * 1024 * 1024
PER_CORE = FULL_ELEMS // N_CORES  # 4_194_304
P = 128
FREE = PER_CORE // P  # 32768 fp32 elements per partition per tensor
THRESH = 0.5
BUFS = 4  # SBUF ring depth per tensor (4 x 16 KiB rows per partition)

# (width, mode); widths sum to FREE. "dve" tiles compute loss2 on vector
# (one extra STT), "act" tiles on scalar (Copy + Sign). The last tile is
# dve so the final tail is a pair of short vector ops.
TILES = (
    (4096, "act"), (4096, "dve"), (4096, "act"), (4096, "dve"),
    (4096, "act"), (4096, "dve"), (4096, "act"),
    (2048, "act"), (1024, "dve"), (1024, "dve"),
)
assert sum(w for w, _ in TILES) == FREE

_CACHE: dict = {}


def _build(n_cores: int):
    f32 = mybir.dt.float32
    nt = len(TILES)
    nc = bacc.Bacc(
        "TRN2", target_bir_lowering=False, debug=False, num_devices=n_cores
    )
    inp = nc.dram_tensor("input", [P * FREE], f32, kind="ExternalInput").ap()
    tgt = nc.dram_tensor("target", [P * FREE], f32, kind="ExternalInput").ap()
    stats = nc.dram_tensor("stats", [P, 3 * nt], f32, kind="ExternalOutput").ap()

    ti_ring = nc.alloc_sbuf_tensor("ti_ring", [P, BUFS * 4096], f32).ap()
    tt_ring = nc.alloc_sbuf_tensor("tt_ring", [P, BUFS * 4096], f32).ap()
    # vector's write-only STT sink lives in PSUM (exactly 4096 fp32/part);
    # self-waits serialize retirement so single-buffer reuse is safe
    sd = nc.alloc_psum_tensor("sd", [P, 4096], f32).ap()
    # scalar alternates two SBUF sinks (deep-pipeline WAW)
    sa = [nc.alloc_sbuf_tensor(f"sa{i}", [P, 4096], f32).ap() for i in range(2)]
    st = nc.alloc_sbuf_tensor("st", [P, 3 * nt], f32).ap()

    offs = []
    off = 0
    for w, _ in TILES:
        offs.append(off)
        off += P * w

    # cumulative consumer-instruction counts through tile t
    V, S = [], []
    v = s = 0
    for w, mode in TILES:
        v += 2 if mode == "dve" else 1
        s += 0 if mode == "dve" else 2
        V.append(v)
        S.append(s)

    def src(t, ap):
        w = TILES[t][0]
        return ap[offs[t] : offs[t] + P * w].rearrange("(p f) -> p f", p=P)

    def ring(t, ring_ap):
        w = TILES[t][0]
        s_ = (t % BUFS) * 4096
        return ring_ap[:, s_ : s_ + w]

    with ExitStack() as ctx:
        slot_sems = [
            ctx.enter_context(nc.semaphore(f"slot_sem{i}")) for i in range(BUFS)
        ]
        vec_sem = ctx.enter_context(nc.semaphore("vec_sem"))
        sc_sem = ctx.enter_context(nc.semaphore("sc_sem"))
        gp_sem = ctx.enter_context(nc.semaphore("gp_sem"))
        out_sem = ctx.enter_context(nc.semaphore("out_sem"))

        # The first BUFS tile-pairs have no dependencies: emit their
        # dma_starts BEFORE the Block so they execute ahead of the block
        # entry barrier and the DMA backend spins up ~1 us earlier. (Only
        # the dependency-free prefix can move: ring-gated loads before the
        # barrier would deadlock against consumers waiting to enter.)
        for t in range(BUFS):
            sem = slot_sems[t % BUFS]
            nc.sync.dma_start(out=ring(t, ti_ring), in_=src(t, inp)).then_inc(
                sem, 16
            )
            nc.sync.dma_start(out=ring(t, tt_ring), in_=src(t, tgt)).then_inc(
                sem, 16
            )

        block = ctx.enter_context(nc.Block())

        @block.gpsimd
        def _(gpsimd):
            gpsimd.memset(st[:], 0.0).then_inc(gp_sem, 1)

        @block.sync
        def _(sync):
            for t, (w, mode) in enumerate(TILES):
                if t < BUFS:
                    continue  # issued above, before the block
                # ring slot reuse: consumers of tile t-BUFS must be done
                sync.wait_ge(vec_sem, V[t - BUFS])
                if S[t - BUFS] > 0:
                    sync.wait_ge(sc_sem, S[t - BUFS])
                sem = slot_sems[t % BUFS]
                sync.dma_start(out=ring(t, ti_ring), in_=src(t, inp)).then_inc(
                    sem, 16
                )
                sync.dma_start(out=ring(t, tt_ring), in_=src(t, tgt)).then_inc(
                    sem, 16
                )
            # sem update on an accum instruction fires at full instruction
            # retirement (incl. the accumulator write-back), so the stats DMAs
            # can depend on the compute sems directly - no fence instructions.
            # Head DMA ships tiles 0..nt-3 (consumed ~3 us before the taper
            # tiles finish); the tail DMA ships the last two triplets.
            sync.wait_ge(vec_sem, V[-3])
            sync.wait_ge(sc_sem, S[-3])
            sync.wait_ge(gp_sem, 1)
            head = 3 * (nt - 2)
            sync.dma_start(out=stats[:, :head], in_=st[:, :head]).then_inc(
                out_sem, 16
            )
            sync.wait_ge(vec_sem, V[-1])
            sync.wait_ge(sc_sem, S[-1])
            sync.dma_start(out=stats[:, head:], in_=st[:, head:]).then_inc(
                out_sem, 16
            )
            # No completion wait on the stats DMAs: the tail [128, 6] lands
            # ~1.5 us after issue, while the NEFF exit semaphore sweep takes
            # ~7 us after the last engine halts - the data is in DRAM long
            # before nrt returns. Skipping the wait takes the DMA-completion
            # semaphore propagation (~1.4 us) off the critical path.

        @block.vector
        def _(vector):
            vector.wait_ge(gp_sem, 1)
            vi = 0
            for t, (w, mode) in enumerate(TILES):
                in_i = ring(t, ti_ring)
                in_t = ring(t, tt_ring)
                vector.wait_ge(slot_sems[t % BUFS], 32 * (t // BUFS + 1))
                ops = [(mybir.AluOpType.mult, 0)]
                if mode == "dve":
                    ops.append((mybir.AluOpType.add, 2))
                for op1, col in ops:
                    if vi >= 1:
                        # single PSUM sink: serialize on retirement
                        vector.wait_ge(vec_sem, vi)
                    vector.scalar_tensor_tensor(
                        out=sd[:, :w],
                        in0=in_i,
                        scalar=THRESH,
                        in1=in_t,
                        op0=mybir.AluOpType.is_gt,
                        op1=op1,
                        accum_out=st[:, 3 * t + col : 3 * t + col + 1],
                    ).then_inc(vec_sem, 1)
                    vi += 1

        @block.scalar
        def _(scalar):
            scalar.wait_ge(gp_sem, 1)
            si = 0
            for t, (w, mode) in enumerate(TILES):
                if mode == "dve":
                    continue
                in_i = ring(t, ti_ring)
                in_t = ring(t, tt_ring)
                scalar.wait_ge(slot_sems[t % BUFS], 32 * (t // BUFS + 1))
                if si >= 2:
                    scalar.wait_ge(sc_sem, si - 1)
                scalar.activation(
                    out=sa[si % 2][:, :w],
                    in_=in_t,
                    func=mybir.ActivationFunctionType.Copy,
                    accum_out=st[:, 3 * t + 2 : 3 * t + 3],
                ).then_inc(sc_sem, 1)
                si += 1
                if si >= 2:
                    scalar.wait_ge(sc_sem, si - 1)
                # Sign(1 - 2x) = -Sign(x - 0.5); bias=1.0 has a pre-registered
                # const AP; host converts the sum to a >0.5 count
                scalar.activation(
                    out=sa[si % 2][:, :w],
                    in_=in_i,
                    func=mybir.ActivationFunctionType.Sign,
                    bias=1.0,
                    scale=-2.0,
                    accum_out=st[:, 3 * t + 1 : 3 * t + 2],
                ).then_inc(sc_sem, 1)
                si += 1

    nc.compile()
    return nc


def _get_nc():
    key = N_CORES
    if key not in _CACHE:
        _CACHE[key] = _build(key)
    return _CACHE[key]


def kernel(input: np.ndarray, target: np.ndarray, **run_kwargs):
    inp = np.asarray(input, dtype=np.float32).reshape(N_CORES, PER_CORE)
    tgt = np.asarray(target, dtype=np.float32).reshape(N_CORES, PER_CORE)

    nc = _get_nc()
    in_maps = [
        {"input": np.ascontiguousarray(inp[c]), "target": np.ascontiguousarray(tgt[c])}
        for c in range(N_CORES)
    ]
    res = run_bass_kernel_spmd(nc, in_maps, core_ids=list(range(N_CORES)), **run_kwargs)

    nt = len(TILES)
    act = [t for t, (_, m) in enumerate(TILES) if m == "act"]
    dve = [t for t, (_, m) in enumerate(TILES) if m == "dve"]
    inter = 0.0
    loss2 = 0.0
    sign_sum = 0.0
    for c in range(N_CORES):
        stats = res.results[c]["stats"].astype(np.float64).reshape(P, nt, 3)
        inter += stats[:, :, 0].sum()
        # dve tiles: col 2 holds direct (bin + tgt) partials
        loss2 += stats[:, dve, 2].sum()
        # act tiles: col 2 holds tgt sums, col 1 holds sign sums
        loss2 += stats[:, act, 2].sum()
        sign_sum += stats[:, act, 1].sum()
    n_act_elems = N_CORES * P * sum(TILES[t][0] for t in act)
    # bin count from sign sums: S' = #lt - #gt -> count(>thr) = (n - S')/2
    loss2 += (n_act_elems - sign_sum) / 2.0

    loss1 = np.float32(2.0 * inter)
    loss2 = np.float32(loss2)
    out = (loss1, loss2)
    if run_kwargs.get("trace"):
        return out, res
    return out
